# revision 49
# baseline (speedup 1.0000x reference)
"""CRF loss (forward-algorithm partition + gold-path score) on 8 Trainium2 cores.

Data-parallel over batch (256/8 = 32 per core). Three independent pieces per
core, engineered so the only serial dependence is a 512-wall-step scan:

1. Partition function: meet-in-the-middle. A forward chain alpha covers
   s = 1..512 (post-multiply form  alpha <- (Ep^T alpha) * w_s) and a backward
   chain beta covers s = 1023..513 (pre-multiply form  v <- w_s * beta,
   beta <- Ep v), both in probability space with Ep = exp(trans) * 2^-9 and a
   column renorm every 64 steps. They meet with one dot:
   Z_b = sum_j alpha[j,b] * beta[j,b]. 512 wall-steps instead of 1023, and the
   per-step PSUM-evacuation multiplies alternate between DVE and GPSIMD so
   neither engine's fixed per-op cost serializes the chain.

2. Gold-path score: no scan at all. The loss only needs batch SUMS, so
   emit_total = trace(EM^T @ MASK) and trans_total = <trans, C> with
   C = sum_n mask_n mask_{n+1}^T, computed as fp8 matmuls over host-relayouted
   one-hot tag masks (row-tiled [128, T] with one-row overlap so every
   consecutive pair is intra-tile), accumulated into two PSUM banks on the
   mostly-idle PE, interleaved one tile per wall-step.

3. Emissions stream: host supplies bf16 [T, S, Bc]; one DMA + one ACT Exp per
   64-step chunk (fwd chunks 0..7 from the left, bwd chunks 15..8 from the
   right).
"""

import sys

import numpy as np

sys.path.insert(0, "/opt/trn_rl_repo")

import ml_dtypes

import concourse.bacc as bacc_mod
import concourse.bass as bass
import concourse.mybir as mybir
import concourse.tile as tile
from concourse.bass_utils import run_bass_kernel_spmd

B, S, T = 256, 1024, 128
NCORES = 8
Bc = B // NCORES  # 32
START, END = T - 2, T - 1  # 126, 127
K = 64            # W chunk size
R = 128           # renorm period
NW = S // K       # 16 chunks
M = S // 2        # meet point: fwd s=1..M, bwd s=S-1..M+1
PRE_BITS = 8.5
ROWS_PER_B = 9 * 128   # 9 overlapping tiles per sequence in the gold streams
NTILES = Bc * 9        # 288 gold tiles per core
NGRP = NTILES // 4     # gold tiles are DMA'd 4 at a time
NREN = M // R - 1      # 3 renorms per direction
F32 = mybir.dt.float32
BF16 = mybir.dt.bfloat16
FP8 = mybir.dt.float8e4
I32 = mybir.dt.int32


def _build_kernel() -> bass.Bass:
    nc = bacc_mod.Bacc()
    emT = nc.dram_tensor("emT", [T, S, Bc], BF16, kind="ExternalInput")
    # packed gold stream: per row [mask fp8 x128 | maskS fp8 x128 | em bf16 x128]
    goldpack_d = nc.dram_tensor("goldpack", [NTILES * 128, 512], mybir.dt.uint8, kind="ExternalInput")
    trans_d = nc.dram_tensor("trans", [T, T], F32, kind="ExternalInput")
    transT_d = nc.dram_tensor("transT", [T, T], F32, kind="ExternalInput")
    pdot_out = nc.dram_tensor("Pdot", [T, Bc], F32, kind="ExternalOutput")
    zv_out = nc.dram_tensor("zv", [1, 2 * NREN * Bc], F32, kind="ExternalOutput")
    gold_out = nc.dram_tensor("gold", [1, 2], F32, kind="ExternalOutput")

    Exp = mybir.ActivationFunctionType.Exp
    Copy = mybir.ActivationFunctionType.Copy
    Ln = mybir.ActivationFunctionType.Ln
    AX = mybir.AxisListType.X
    Alu = mybir.AluOpType
    BIAS0 = float(-PRE_BITS * np.log(2.0))

    with tile.TileContext(nc) as tc:
        with (
            tc.tile_pool(name="constp", bufs=1) as constp,
            tc.tile_pool(name="wp", bufs=3) as wp,
            tc.tile_pool(name="goldp", bufs=4) as goldp,
            tc.tile_pool(name="statep", bufs=3) as statep,
            tc.tile_pool(name="miscp", bufs=1) as miscp,
            tc.tile_pool(name="psq", bufs=2, space="PSUM") as psq,
            tc.tile_pool(name="psacc", bufs=1, space="PSUM") as psacc,
            tc.tile_pool(name="psz", bufs=1, space="PSUM") as psz,
        ):
            # ---- first W chunk DMAs go out before everything else: the first
            # scan multiply is gated on exp(chunk0), so those transfers lead
            # the serialized DMA issue path.
            raw_first = {}
            for c, side in ((0, "f"), (NW - 1, "b")):
                rawt = wp.tile([T, K * Bc], BF16, tag=f"raw{side}", bufs=4)
                nc.sync.dma_start(
                    out=rawt[:].rearrange("t (s b) -> t s b", s=K),
                    in_=emT[:, c * K : (c + 1) * K, :],
                )
                raw_first[c] = rawt

            # ---- constants ----
            trans_t = constp.tile([T, T], F32)
            nc.sync.dma_start(out=trans_t[:], in_=trans_d[:, :])
            bias0_t = constp.tile([T, 1], F32)
            nc.vector.memset(bias0_t[:], BIAS0)
            zero_t = constp.tile([T, 1], F32)
            nc.vector.memset(zero_t[:], 0.0)
            Ep = constp.tile([T, T], BF16)          # exp(trans) * 2^-9
            nc.scalar.activation(Ep[:], trans_t[:], Exp, bias=bias0_t[:])
            ones_bf = constp.tile([T, T], BF16)
            nc.vector.memset(ones_bf[:], 1.0)
            ones_f32 = constp.tile([T, 1], F32)
            nc.vector.memset(ones_f32[:], 1.0)

            pid = constp.tile([T, 1], I32)
            nc.gpsimd.iota(pid[:], pattern=[[0, 1]], base=0, channel_multiplier=1)
            fid = constp.tile([T, T], I32)
            nc.gpsimd.iota(fid[:], pattern=[[1, T]], base=0, channel_multiplier=0)
            ident = constp.tile([T, T], BF16)
            nc.vector.tensor_tensor(
                out=ident[:], in0=pid[:].to_broadcast([T, T]), in1=fid[:], op=Alu.is_equal
            )
            # EpT = exp(trans^T) * 2^-9: backward-chain lhsT (out = Ep @ rhs),
            # built from the host-transposed copy of the input.
            transT_t = constp.tile([T, T], F32)
            nc.sync.dma_start(out=transT_t[:], in_=transT_d[:, :])
            EpT = constp.tile([T, T], BF16)
            nc.scalar.activation(EpT[:], transT_t[:], Exp, bias=bias0_t[:])

            # ---- W chunk machinery ----
            # chunk c covers s in [64c, 64c+64); fwd consumes chunks 0..7
            # (slices s%64 = 1..63 of chunk m plus slice 0 of chunk m+1), bwd
            # consumes chunks 15..8 top-down. Chunk 8's slice 0 (s=512) is the
            # final fwd step.
            wtiles: dict[int, object] = {}

            def load_chunk(c: int, side: str):
                raw = raw_first.pop(c, None)
                if raw is None:
                    raw = wp.tile([T, K * Bc], BF16, tag=f"raw{side}", bufs=4)
                    nc.sync.dma_start(
                        out=raw[:].rearrange("t (s b) -> t s b", s=K),
                        in_=emT[:, c * K : (c + 1) * K, :],
                    )
                w = wp.tile([T, K * Bc], BF16, tag=f"w{side}")
                nc.scalar.activation(w[:], raw[:], Exp, bias=zero_t[:])
                wtiles[c] = w

            # ---- gold stream machinery: 16 packed tiles per DMA group ----
            GT = 16
            GOFF = 48  # first wall-step that runs gold matmuls
            def load_gold_group(g: int):
                gb = goldp.tile([T, GT * 512], mybir.dt.uint8, tag="gb")
                nc.scalar.dma_start(
                    out=gb[:].rearrange("p (j c) -> p j c", j=GT),
                    in_=goldpack_d[g * GT * 128 : (g + 1) * GT * 128, :].rearrange(
                        "(j p) c -> p j c", p=128
                    ),
                )
                return gb

            # ---- init states ----
            alpha = statep.tile([T, Bc], BF16, tag="alpha")
            nc.vector.tensor_scalar(
                out=alpha[:], in0=pid[:].to_broadcast([T, Bc]),
                scalar1=START, scalar2=None, op0=Alu.is_equal,
            )
            zbuf = miscp.tile([1, 2 * NREN * Bc], F32)

            def renorm(st, slot):
                """Column-renormalize st (SBUF bf16 [T,Bc]): PE replicated
                column sums, DVE reciprocal, Pool scale (SBUF-only). The raw z
                row goes to zbuf; the ln happens on host."""
                zb = psz.tile([T, Bc], F32, tag="zb", bufs=2)
                nc.tensor.matmul(out=zb[:], lhsT=ones_bf[:], rhs=st[:], start=True, stop=True)
                zrec = statep.tile([T, Bc], F32, tag="zrec", bufs=2)
                nc.vector.reciprocal(out=zrec[:], in_=zb[:])
                stn = statep.tile([T, Bc], BF16, tag="renst", bufs=2)
                nc.gpsimd.tensor_mul(out=stn[:], in0=st[:], in1=zrec[:])
                nc.scalar.copy(
                    out=zbuf[:, slot * Bc : (slot + 1) * Bc], in_=zb[0:1, :]
                )
                return stn

            Dacc = psacc.tile([T, T], F32, tag="D")
            Cacc = psacc.tile([T, T], F32, tag="C")

            # prologue: first chunks + first three gold groups (the gold
            # stream is prefetched three groups ahead so its DMA never gates
            # the PE)
            load_chunk(0, "f")
            load_chunk(NW - 1, "b")
            gold_q = [load_gold_group(0), load_gold_group(1), load_gold_group(2)]

            vb = None          # bwd pre-multiplied state (SBUF bf16)
            beta_ps = None     # bwd matmul output (PSUM f32)

            for k in range(M):
                win, sl = divmod(k, K)
                if sl == 0:
                    # prefetch: fwd needs chunk win+1 (for its slice 0 at
                    # k = 64*win+63); bwd needs chunk 14-win for next window.
                    if win + 1 <= 7:
                        load_chunk(win + 1, "f")
                    if win < 7:
                        load_chunk(NW - 2 - win, "b")

                s_f = k + 1
                wf = wtiles[s_f // K]
                cols_f = slice((s_f % K) * Bc, (s_f % K + 1) * Bc)
                s_b = S - 1 - k
                wb = wtiles[s_b // K]
                cols_b = slice((s_b % K) * Bc, (s_b % K + 1) * Bc)

                is_ren = k % R == R - 1 and k != M - 1

                # forward: qf = Ep^T alpha ; alpha' = wf_s * qf
                qf = psq.tile([T, Bc], F32, tag="qf")
                nc.tensor.matmul(out=qf[:], lhsT=Ep[:], rhs=alpha[:], start=True, stop=True)
                alpha_n = statep.tile([T, Bc], BF16, tag="alpha")
                nc.vector.tensor_mul(out=alpha_n[:], in0=wf[:, cols_f], in1=qf[:])
                alpha = renorm(alpha_n, 2 * (k // R)) if is_ren else alpha_n

                # gold: one packed tile (2 matmuls) per wall-step, starting at
                # GOFF so prologue DMAs never gate the PE queue. Emitted here
                # -- after this step's fwd matmul, before the bwd matmul -- so
                # they fill PE's idle window while DVE runs the multiplies.
                t = k - GOFF
                if 0 <= t < NTILES:
                    g, j = divmod(t, GT)
                    gb = gold_q[0]
                    mk = gb[:, j * 512 : j * 512 + 128].bitcast(FP8)
                    sk = gb[:, j * 512 + 128 : j * 512 + 256].bitcast(FP8)
                    ek = gb[:, j * 512 + 256 : j * 512 + 512].bitcast(BF16)
                    nc.tensor.matmul(
                        out=Dacc[:], lhsT=ek, rhs=mk,
                        start=(t == 0), stop=(t == NTILES - 1),
                    )
                    nc.tensor.matmul(
                        out=Cacc[:], lhsT=mk, rhs=sk,
                        start=(t == 0), stop=(t == NTILES - 1),
                    )
                    if j == GT - 1 and g + 1 < NTILES // GT:
                        gold_q.pop(0)
                        if g + 3 < NTILES // GT:
                            gold_q.append(load_gold_group(g + 3))

                # backward: v = wb_s * beta ; beta' = Ep v
                # (bwd matmuls at k=0..M-2 produce beta_1023..beta_513; no bwd
                # work at k=M-1 -- the final beta_513 PSUM feeds the meet dot.)
                if k == 0:
                    rhs_b = wb[:, cols_b]  # v = w_1023 * ones
                elif k < M - 1:
                    vb_n = statep.tile([T, Bc], BF16, tag="vb")
                    nc.vector.tensor_mul(out=vb_n[:], in0=wb[:, cols_b], in1=beta_ps)
                    vb = renorm(vb_n, 2 * (k // R) + 1) if is_ren else vb_n
                    rhs_b = vb[:]
                if k < M - 1:
                    qb = psq.tile([T, Bc], F32, tag="qb")
                    nc.tensor.matmul(out=qb[:], lhsT=EpT[:], rhs=rhs_b, start=True, stop=True)
                    beta_ps = qb[:]

            # ---- finalize partition: Z_b = sum_j alpha[j,b] * beta_513[j,b].
            # The elementwise product and the renorm logs go out raw; the
            # 128-way sum + ln + adds are host post-processing (the on-device
            # reduction hit an execute-path PSUM corruption; this is robust).
            P = statep.tile([T, Bc], F32, tag="dotP")
            nc.vector.tensor_mul(out=P[:], in0=alpha[:], in1=beta_ps)
            nc.sync.dma_start(out=pdot_out[:, :], in_=P[:])
            nc.sync.dma_start(out=zv_out[:, :], in_=zbuf[:])

            # ---- finalize gold: emit = tr(D), trans = <trans, C> ----
            gold = miscp.tile([1, 2], F32)
            for idx, (acc, weight) in enumerate(((Dacc, ident), (Cacc, trans_t))):
                tmp = miscp.tile([T, T], F32, tag=f"gt{idx}")
                nc.vector.tensor_mul(out=tmp[:], in0=weight[:], in1=acc[:])
                col = miscp.tile([T, 1], F32, tag=f"gc{idx}")
                nc.vector.reduce_sum(out=col[:], in_=tmp[:], axis=AX)
                tot = psz.tile([T, Bc], F32, tag="zb", bufs=2)
                nc.tensor.matmul(
                    out=tot[0:1, 0:1], lhsT=ones_f32[:], rhs=col[:], start=True, stop=True
                )
                nc.vector.tensor_copy(out=gold[:, idx : idx + 1], in_=tot[0:1, 0:1])
            nc.sync.dma_start(out=gold_out[:, :], in_=gold[:])

    nc.compile()
    return nc


def _make_gold_streams(em_core: np.ndarray, tags_core: np.ndarray):
    """Host relayout: overlapping 128-row tiles of the one-hot mask / emission
    streams. Per sequence b: logical rows 0..1025 are [start, tags, end]
    one-hots (mask) / [0, em rows, 0] (em); tile t covers logical rows
    127t..127t+127 so every consecutive pair is intra-tile. The overlap row is
    duplicated in the mask stream and zeroed in the em stream (tile t carries
    em for logical rows 127t..127t+126 only)."""
    maskL = np.zeros((Bc, 1026, T), dtype=np.float32)
    bidx = np.arange(Bc)[:, None]
    maskL[:, 0, START] = 1.0
    maskL[bidx, 1 + np.arange(S)[None, :], tags_core] = 1.0
    maskL[:, 1025, END] = 1.0
    emL = np.zeros((Bc, 1026, T), dtype=np.float32)
    emL[:, 1 : S + 1, :] = em_core

    maskTiles = np.zeros((Bc, 9, 128, T), dtype=np.float32)
    maskShift = np.zeros((Bc, 9, 128, T), dtype=np.float32)
    emTiles = np.zeros((Bc, 9, 128, T), dtype=np.float32)
    for t in range(9):
        lo = 127 * t
        n = min(128, 1026 - lo)
        maskTiles[:, t, :n] = maskL[:, lo : lo + n]
        # shift stream: row p = maskL[lo+p+1], rows 0..126 only (row 127 = 0),
        # so tile t contributes exactly the pairs (lo+p, lo+p+1), p = 0..126.
        ns = min(127, 1025 - lo)
        maskShift[:, t, :ns] = maskL[:, lo + 1 : lo + 1 + ns]
        ne = min(127, 1026 - lo)
        emTiles[:, t, :ne] = emL[:, lo : lo + ne]
    mk = maskTiles.reshape(NTILES * 128, T).astype(ml_dtypes.float8_e4m3fn)
    sk = maskShift.reshape(NTILES * 128, T).astype(ml_dtypes.float8_e4m3fn)
    ek = emTiles.reshape(NTILES * 128, T).astype(ml_dtypes.bfloat16)
    return np.concatenate(
        [mk.view(np.uint8), sk.view(np.uint8), ek.view(np.uint8)], axis=1
    )


_NC_CACHE: list = []


def kernel(emissions: np.ndarray, tags: np.ndarray, transitions: np.ndarray) -> np.ndarray:
    emissions = np.asarray(emissions, dtype=np.float32)
    tags_np = np.asarray(tags).astype(np.int64)
    transitions = np.ascontiguousarray(np.asarray(transitions, dtype=np.float32))

    if not _NC_CACHE:
        _NC_CACHE.append(_build_kernel())
    nc = _NC_CACHE[0]

    in_maps = []
    for c in range(NCORES):
        sl = slice(c * Bc, (c + 1) * Bc)
        em_core = emissions[sl]  # [Bc, S, T]
        in_maps.append(
            {
                "emT": np.ascontiguousarray(
                    em_core.transpose(2, 1, 0).astype(ml_dtypes.bfloat16)
                ),
                "goldpack": _make_gold_streams(em_core, tags_np[sl]),
                "trans": transitions,
                "transT": np.ascontiguousarray(transitions.T),
            }
        )

    kernel._last_in_maps = in_maps
    results = run_bass_kernel_spmd(nc, in_maps, core_ids=list(range(NCORES))).results

    const = np.float64((S - 1) * PRE_BITS * np.log(2.0) - 10000.0)
    total = np.float64(0.0)
    for c in range(NCORES):
        r = results[c]
        dot = r["Pdot"].astype(np.float64).sum(axis=0)  # [Bc]
        lnz = np.log(r["zv"].reshape(2 * NREN, Bc).astype(np.float64)).sum(axis=0)
        part = np.log(dot) + lnz + const
        emit_tot, trans_tot = r["gold"].reshape(-1).astype(np.float64)
        total += part.sum() - emit_tot - trans_tot

    return np.array(total / B, dtype=np.float32)


# revision 50
# speedup vs baseline: 1.0168x; 1.0168x over previous
"""CRF loss (forward-algorithm partition + gold-path score) on 8 Trainium2 cores.

Data-parallel over batch (256/8 = 32 per core). Three independent pieces per
core, engineered so the only serial dependence is a 512-wall-step scan:

1. Partition function: meet-in-the-middle. A forward chain alpha covers
   s = 1..512 (post-multiply form  alpha <- (Ep^T alpha) * w_s) and a backward
   chain beta covers s = 1023..513 (pre-multiply form  v <- w_s * beta,
   beta <- Ep v), both in probability space with Ep = exp(trans) * 2^-9 and a
   column renorm every 64 steps. They meet with one dot:
   Z_b = sum_j alpha[j,b] * beta[j,b]. 512 wall-steps instead of 1023, and the
   per-step PSUM-evacuation multiplies alternate between DVE and GPSIMD so
   neither engine's fixed per-op cost serializes the chain.

2. Gold-path score: no scan at all. The loss only needs batch SUMS, so
   emit_total = trace(EM^T @ MASK) and trans_total = <trans, C> with
   C = sum_n mask_n mask_{n+1}^T, computed as fp8 matmuls over host-relayouted
   one-hot tag masks (row-tiled [128, T] with one-row overlap so every
   consecutive pair is intra-tile), accumulated into two PSUM banks on the
   mostly-idle PE, interleaved one tile per wall-step.

3. Emissions stream: host supplies bf16 [T, S, Bc]; one DMA + one ACT Exp per
   64-step chunk (fwd chunks 0..7 from the left, bwd chunks 15..8 from the
   right).
"""

import sys

import numpy as np

sys.path.insert(0, "/opt/trn_rl_repo")

import ml_dtypes

import concourse.bacc as bacc_mod
import concourse.bass as bass
import concourse.mybir as mybir
import concourse.tile as tile
from concourse.bass_utils import run_bass_kernel_spmd

B, S, T = 256, 1024, 128
NCORES = 8
Bc = B // NCORES  # 32
START, END = T - 2, T - 1  # 126, 127
K = 64            # W chunk size
R = 128           # renorm period
NW = S // K       # 16 chunks
M = S // 2        # meet point: fwd s=1..M, bwd s=S-1..M+1
PRE_BITS = 8.5
ROWS_PER_B = 9 * 128   # 9 overlapping tiles per sequence in the gold streams
NTILES = Bc * 9        # 288 gold tiles per core
NGRP = NTILES // 4     # gold tiles are DMA'd 4 at a time
NREN = M // R - 1      # 3 renorms per direction
F32 = mybir.dt.float32
BF16 = mybir.dt.bfloat16
FP8 = mybir.dt.float8e4
I32 = mybir.dt.int32


def _build_kernel() -> bass.Bass:
    nc = bacc_mod.Bacc()
    emT = nc.dram_tensor("emT", [T, S, Bc], BF16, kind="ExternalInput")
    # packed gold stream: per row [mask fp8 x128 | maskS fp8 x128 | em bf16 x128]
    goldpack_d = nc.dram_tensor("goldpack", [NTILES * 128, 512], mybir.dt.uint8, kind="ExternalInput")
    trans_d = nc.dram_tensor("trans", [T, T], F32, kind="ExternalInput")
    transT_d = nc.dram_tensor("transT", [T, T], F32, kind="ExternalInput")
    pdot_out = nc.dram_tensor("Pdot", [T, Bc], F32, kind="ExternalOutput")
    zv_out = nc.dram_tensor("zv", [1, 2 * NREN * Bc], F32, kind="ExternalOutput")
    gold_out = nc.dram_tensor("gold", [1, 2], F32, kind="ExternalOutput")

    Exp = mybir.ActivationFunctionType.Exp
    Copy = mybir.ActivationFunctionType.Copy
    Ln = mybir.ActivationFunctionType.Ln
    AX = mybir.AxisListType.X
    Alu = mybir.AluOpType
    BIAS0 = float(-PRE_BITS * np.log(2.0))

    with tile.TileContext(nc) as tc:
        with (
            tc.tile_pool(name="constp", bufs=1) as constp,
            tc.tile_pool(name="wp", bufs=3) as wp,
            tc.tile_pool(name="goldp", bufs=3) as goldp,
            tc.tile_pool(name="statep", bufs=3) as statep,
            tc.tile_pool(name="miscp", bufs=1) as miscp,
            tc.tile_pool(name="psq", bufs=2, space="PSUM") as psq,
            tc.tile_pool(name="psacc", bufs=1, space="PSUM") as psacc,
            tc.tile_pool(name="psz", bufs=1, space="PSUM") as psz,
        ):
            # ---- constants ----
            trans_t = constp.tile([T, T], F32)
            nc.sync.dma_start(out=trans_t[:], in_=trans_d[:, :])
            bias0_t = constp.tile([T, 1], F32)
            nc.vector.memset(bias0_t[:], BIAS0)
            zero_t = constp.tile([T, 1], F32)
            nc.vector.memset(zero_t[:], 0.0)
            Ep = constp.tile([T, T], BF16)          # exp(trans) * 2^-9
            nc.scalar.activation(Ep[:], trans_t[:], Exp, bias=bias0_t[:])
            ones_bf = constp.tile([T, T], BF16)
            nc.vector.memset(ones_bf[:], 1.0)
            ones_f32 = constp.tile([T, 1], F32)
            nc.vector.memset(ones_f32[:], 1.0)

            pid = constp.tile([T, 1], I32)
            nc.gpsimd.iota(pid[:], pattern=[[0, 1]], base=0, channel_multiplier=1)
            fid = constp.tile([T, T], I32)
            nc.gpsimd.iota(fid[:], pattern=[[1, T]], base=0, channel_multiplier=0)
            ident = constp.tile([T, T], BF16)
            nc.vector.tensor_tensor(
                out=ident[:], in0=pid[:].to_broadcast([T, T]), in1=fid[:], op=Alu.is_equal
            )
            # EpT = exp(trans^T) * 2^-9: backward-chain lhsT (out = Ep @ rhs),
            # built from the host-transposed copy of the input.
            transT_t = constp.tile([T, T], F32)
            nc.sync.dma_start(out=transT_t[:], in_=transT_d[:, :])
            EpT = constp.tile([T, T], BF16)
            nc.scalar.activation(EpT[:], transT_t[:], Exp, bias=bias0_t[:])

            # ---- W chunk machinery ----
            # chunk c covers s in [64c, 64c+64); fwd consumes chunks 0..7
            # (slices s%64 = 1..63 of chunk m plus slice 0 of chunk m+1), bwd
            # consumes chunks 15..8 top-down. Chunk 8's slice 0 (s=512) is the
            # final fwd step.
            wtiles: dict[int, object] = {}

            def load_chunk(c: int, side: str):
                raw = wp.tile([T, K * Bc], BF16, tag=f"raw{side}", bufs=4)
                nc.sync.dma_start(
                    out=raw[:].rearrange("t (s b) -> t s b", s=K),
                    in_=emT[:, c * K : (c + 1) * K, :],
                )
                w = wp.tile([T, K * Bc], BF16, tag=f"w{side}")
                nc.scalar.activation(w[:], raw[:], Exp, bias=zero_t[:])
                wtiles[c] = w

            # ---- gold stream machinery: 16 packed tiles per DMA group ----
            GT = 16
            GOFF = 48  # first wall-step that runs gold matmuls
            def load_gold_group(g: int):
                gb = goldp.tile([T, GT * 512], mybir.dt.uint8, tag="gb")
                nc.scalar.dma_start(
                    out=gb[:].rearrange("p (j c) -> p j c", j=GT),
                    in_=goldpack_d[g * GT * 128 : (g + 1) * GT * 128, :].rearrange(
                        "(j p) c -> p j c", p=128
                    ),
                )
                return gb

            # ---- init states ----
            alpha = statep.tile([T, Bc], BF16, tag="alpha")
            nc.vector.tensor_scalar(
                out=alpha[:], in0=pid[:].to_broadcast([T, Bc]),
                scalar1=START, scalar2=None, op0=Alu.is_equal,
            )
            zbuf = miscp.tile([1, 2 * NREN * Bc], F32)

            def renorm(st, slot):
                """Column-renormalize st (SBUF bf16 [T,Bc]): PE replicated
                column sums, DVE reciprocal, Pool scale (SBUF-only). The raw z
                row goes to zbuf; the ln happens on host."""
                zb = psz.tile([T, Bc], F32, tag="zb", bufs=2)
                nc.tensor.matmul(out=zb[:], lhsT=ones_bf[:], rhs=st[:], start=True, stop=True)
                zrec = statep.tile([T, Bc], F32, tag="zrec", bufs=2)
                nc.vector.reciprocal(out=zrec[:], in_=zb[:])
                stn = statep.tile([T, Bc], BF16, tag="renst", bufs=2)
                nc.gpsimd.tensor_mul(out=stn[:], in0=st[:], in1=zrec[:])
                nc.scalar.copy(
                    out=zbuf[:, slot * Bc : (slot + 1) * Bc], in_=zb[0:1, :]
                )
                return stn

            Dacc = psacc.tile([T, T], F32, tag="D")
            Cacc = psacc.tile([T, T], F32, tag="C")

            # prologue: first chunks + first two gold groups (the gold stream
            # is prefetched two groups ahead so its DMA never gates the PE)
            load_chunk(0, "f")
            load_chunk(NW - 1, "b")
            gold_tiles = load_gold_group(0)
            gold_next = load_gold_group(1)

            vb = None          # bwd pre-multiplied state (SBUF bf16)
            beta_ps = None     # bwd matmul output (PSUM f32)

            for k in range(M):
                win, sl = divmod(k, K)
                if sl == 0:
                    # prefetch: fwd needs chunk win+1 (for its slice 0 at
                    # k = 64*win+63); bwd needs chunk 14-win for next window.
                    if win + 1 <= 7:
                        load_chunk(win + 1, "f")
                    if win < 7:
                        load_chunk(NW - 2 - win, "b")

                s_f = k + 1
                wf = wtiles[s_f // K]
                cols_f = slice((s_f % K) * Bc, (s_f % K + 1) * Bc)
                s_b = S - 1 - k
                wb = wtiles[s_b // K]
                cols_b = slice((s_b % K) * Bc, (s_b % K + 1) * Bc)

                is_ren = k % R == R - 1 and k != M - 1

                # forward: qf = Ep^T alpha ; alpha' = wf_s * qf
                qf = psq.tile([T, Bc], F32, tag="qf")
                nc.tensor.matmul(out=qf[:], lhsT=Ep[:], rhs=alpha[:], start=True, stop=True)
                alpha_n = statep.tile([T, Bc], BF16, tag="alpha")
                nc.vector.tensor_mul(out=alpha_n[:], in0=wf[:, cols_f], in1=qf[:])
                alpha = renorm(alpha_n, 2 * (k // R)) if is_ren else alpha_n

                # gold: one packed tile (2 matmuls) per wall-step, starting at
                # GOFF so prologue DMAs never gate the PE queue. Emitted here
                # -- after this step's fwd matmul, before the bwd matmul -- so
                # they fill PE's idle window while DVE runs the multiplies.
                t = k - GOFF
                if 0 <= t < NTILES:
                    g, j = divmod(t, GT)
                    gb = gold_tiles
                    mk = gb[:, j * 512 : j * 512 + 128].bitcast(FP8)
                    sk = gb[:, j * 512 + 128 : j * 512 + 256].bitcast(FP8)
                    ek = gb[:, j * 512 + 256 : j * 512 + 512].bitcast(BF16)
                    nc.tensor.matmul(
                        out=Dacc[:], lhsT=ek, rhs=mk,
                        start=(t == 0), stop=(t == NTILES - 1),
                    )
                    nc.tensor.matmul(
                        out=Cacc[:], lhsT=mk, rhs=sk,
                        start=(t == 0), stop=(t == NTILES - 1),
                    )
                    if j == GT - 1 and g + 1 < NTILES // GT:
                        gold_tiles = gold_next
                        if g + 2 < NTILES // GT:
                            gold_next = load_gold_group(g + 2)

                # backward: v = wb_s * beta ; beta' = Ep v
                # (bwd matmuls at k=0..M-2 produce beta_1023..beta_513; no bwd
                # work at k=M-1 -- the final beta_513 PSUM feeds the meet dot.)
                if k == 0:
                    rhs_b = wb[:, cols_b]  # v = w_1023 * ones
                elif k < M - 1:
                    vb_n = statep.tile([T, Bc], BF16, tag="vb")
                    nc.vector.tensor_mul(out=vb_n[:], in0=wb[:, cols_b], in1=beta_ps)
                    vb = renorm(vb_n, 2 * (k // R) + 1) if is_ren else vb_n
                    rhs_b = vb[:]
                if k < M - 1:
                    qb = psq.tile([T, Bc], F32, tag="qb")
                    nc.tensor.matmul(out=qb[:], lhsT=EpT[:], rhs=rhs_b, start=True, stop=True)
                    beta_ps = qb[:]

            # ---- finalize partition: Z_b = sum_j alpha[j,b] * beta_513[j,b].
            # The elementwise product and the renorm logs go out raw; the
            # 128-way sum + ln + adds are host post-processing (the on-device
            # reduction hit an execute-path PSUM corruption; this is robust).
            P = statep.tile([T, Bc], F32, tag="dotP")
            nc.vector.tensor_mul(out=P[:], in0=alpha[:], in1=beta_ps)
            nc.sync.dma_start(out=pdot_out[:, :], in_=P[:])
            nc.sync.dma_start(out=zv_out[:, :], in_=zbuf[:])

            # ---- finalize gold: emit = tr(D), trans = <trans, C> ----
            gold = miscp.tile([1, 2], F32)
            for idx, (acc, weight) in enumerate(((Dacc, ident), (Cacc, trans_t))):
                tmp = miscp.tile([T, T], F32, tag=f"gt{idx}")
                nc.vector.tensor_mul(out=tmp[:], in0=weight[:], in1=acc[:])
                col = miscp.tile([T, 1], F32, tag=f"gc{idx}")
                nc.vector.reduce_sum(out=col[:], in_=tmp[:], axis=AX)
                tot = psz.tile([T, Bc], F32, tag="zb", bufs=2)
                nc.tensor.matmul(
                    out=tot[0:1, 0:1], lhsT=ones_f32[:], rhs=col[:], start=True, stop=True
                )
                nc.vector.tensor_copy(out=gold[:, idx : idx + 1], in_=tot[0:1, 0:1])
            nc.sync.dma_start(out=gold_out[:, :], in_=gold[:])

    nc.compile()
    return nc


def _make_gold_streams(em_core: np.ndarray, tags_core: np.ndarray):
    """Host relayout: overlapping 128-row tiles of the one-hot mask / emission
    streams. Per sequence b: logical rows 0..1025 are [start, tags, end]
    one-hots (mask) / [0, em rows, 0] (em); tile t covers logical rows
    127t..127t+127 so every consecutive pair is intra-tile. The overlap row is
    duplicated in the mask stream and zeroed in the em stream (tile t carries
    em for logical rows 127t..127t+126 only)."""
    maskL = np.zeros((Bc, 1026, T), dtype=np.float32)
    bidx = np.arange(Bc)[:, None]
    maskL[:, 0, START] = 1.0
    maskL[bidx, 1 + np.arange(S)[None, :], tags_core] = 1.0
    maskL[:, 1025, END] = 1.0
    emL = np.zeros((Bc, 1026, T), dtype=np.float32)
    emL[:, 1 : S + 1, :] = em_core

    maskTiles = np.zeros((Bc, 9, 128, T), dtype=np.float32)
    maskShift = np.zeros((Bc, 9, 128, T), dtype=np.float32)
    emTiles = np.zeros((Bc, 9, 128, T), dtype=np.float32)
    for t in range(9):
        lo = 127 * t
        n = min(128, 1026 - lo)
        maskTiles[:, t, :n] = maskL[:, lo : lo + n]
        # shift stream: row p = maskL[lo+p+1], rows 0..126 only (row 127 = 0),
        # so tile t contributes exactly the pairs (lo+p, lo+p+1), p = 0..126.
        ns = min(127, 1025 - lo)
        maskShift[:, t, :ns] = maskL[:, lo + 1 : lo + 1 + ns]
        ne = min(127, 1026 - lo)
        emTiles[:, t, :ne] = emL[:, lo : lo + ne]
    mk = maskTiles.reshape(NTILES * 128, T).astype(ml_dtypes.float8_e4m3fn)
    sk = maskShift.reshape(NTILES * 128, T).astype(ml_dtypes.float8_e4m3fn)
    ek = emTiles.reshape(NTILES * 128, T).astype(ml_dtypes.bfloat16)
    return np.concatenate(
        [mk.view(np.uint8), sk.view(np.uint8), ek.view(np.uint8)], axis=1
    )


_NC_CACHE: list = []


def kernel(emissions: np.ndarray, tags: np.ndarray, transitions: np.ndarray) -> np.ndarray:
    emissions = np.asarray(emissions, dtype=np.float32)
    tags_np = np.asarray(tags).astype(np.int64)
    transitions = np.ascontiguousarray(np.asarray(transitions, dtype=np.float32))

    if not _NC_CACHE:
        _NC_CACHE.append(_build_kernel())
    nc = _NC_CACHE[0]

    in_maps = []
    for c in range(NCORES):
        sl = slice(c * Bc, (c + 1) * Bc)
        em_core = emissions[sl]  # [Bc, S, T]
        in_maps.append(
            {
                "emT": np.ascontiguousarray(
                    em_core.transpose(2, 1, 0).astype(ml_dtypes.bfloat16)
                ),
                "goldpack": _make_gold_streams(em_core, tags_np[sl]),
                "trans": transitions,
                "transT": np.ascontiguousarray(transitions.T),
            }
        )

    kernel._last_in_maps = in_maps
    results = run_bass_kernel_spmd(nc, in_maps, core_ids=list(range(NCORES))).results

    const = np.float64((S - 1) * PRE_BITS * np.log(2.0) - 10000.0)
    total = np.float64(0.0)
    for c in range(NCORES):
        r = results[c]
        dot = r["Pdot"].astype(np.float64).sum(axis=0)  # [Bc]
        lnz = np.log(r["zv"].reshape(2 * NREN, Bc).astype(np.float64)).sum(axis=0)
        part = np.log(dot) + lnz + const
        emit_tot, trans_tot = r["gold"].reshape(-1).astype(np.float64)
        total += part.sum() - emit_tot - trans_tot

    return np.array(total / B, dtype=np.float32)
